# revision 15
# baseline (speedup 1.0000x reference)
"""Trainium2 Bass kernel for nn_Conv1DTraining (one SGD step of a pooled
1-D conv + BCE head).

Math (collapsing the reference):
  c = conv_full(kernel, ones(out_len))            # (L,) — constant s=sum(kernel)
                                                  # except 127 edge cols each side
  pooled[b] = sum_j x[b,j]*c[j] = s*T[b] + edge corrections, T[b]=row sum
  u[b]      = (sigmoid(pooled[b]/out_len + bias) - y[b]) / out_len
  w[j]      = sum_b u[b]*x[b,j]                   # the only other O(B*L) pass
  d_kernel[k] = window_sum(w, k..k+out_len-1)/B ; d_bias = out_len*sum(u)/B

Sharding: data-parallel over batch (8192 -> 8 x 1024). Each core streams its
16 MiB x-shard once (DMA-bound), computing T on ACT/DVE (fused f32->bf16 cast
+ row-sum via accum_out) and w on the PE (u^T @ X, bf16, PSUM-accumulated).
Tiny per-core partials (w: 16 KB, u: 4 KB) are reduced on the host, where the
window sums and the SGD update are trivial.
"""

import numpy as np

KERNEL_SIZE = 128
SEQ_LEN = 4096
BATCH = 8192
OUT_LEN = SEQ_LEN - KERNEL_SIZE + 1  # 3969
LR = 1.0
N_CORES = 8
B_SHARD = BATCH // N_CORES  # 1024
N_CHUNKS = B_SHARD // 128  # 8
EDGE = KERNEL_SIZE - 1  # 127
HALF = SEQ_LEN // 2  # 2048

_NC_CACHE = {}


def _build_nc():
    import concourse.bacc as bacc
    import concourse.tile as tile
    from concourse import mybir

    f32 = mybir.dt.float32
    bf16 = mybir.dt.bfloat16
    Alu = mybir.AluOpType
    Act = mybir.ActivationFunctionType

    nc = bacc.Bacc(None, target_bir_lowering=False)

    x = nc.dram_tensor("x", [B_SHARD, SEQ_LEN], f32, kind="ExternalInput")
    y_t = nc.dram_tensor("y_t", [128, N_CHUNKS], f32, kind="ExternalInput")
    cd = nc.dram_tensor("cd", [128, 2 * EDGE], f32, kind="ExternalInput")
    par = nc.dram_tensor("par", [128, 2], f32, kind="ExternalInput")
    w_out = nc.dram_tensor("w_out", [1, SEQ_LEN], f32, kind="ExternalOutput")
    u_out = nc.dram_tensor("u_out", [128, N_CHUNKS], bf16, kind="ExternalOutput")

    with tile.TileContext(nc) as tc:
        with (
            tc.tile_pool(name="singles", bufs=1) as singles,
            tc.tile_pool(name="xpool", bufs=3) as xpool,
            tc.tile_pool(name="bfpool", bufs=3) as bfpool,
            tc.tile_pool(name="small", bufs=4) as small,
            tc.tile_pool(name="psum", bufs=1, space="PSUM") as psum_pool,
        ):
            # Constants ride SWDGE (gpsimd) so the Sync HWDGE ring is
            # dedicated to the big x stream from instruction 0.
            y_sb = singles.tile([128, N_CHUNKS], f32)
            nc.gpsimd.dma_start(out=y_sb[:], in_=y_t[:])
            cd_sb = singles.tile([128, 2 * EDGE], f32)
            nc.gpsimd.dma_start(out=cd_sb[:], in_=cd[:])
            par_sb = singles.tile([128, 2], f32)
            nc.gpsimd.dma_start(out=par_sb[:], in_=par[:])
            # u in bf16 doubles as the matmul stationary operand; the host
            # only needs sum(u), where bf16 noise is ~0.4% of a gradient
            # that is itself ~5% of bias -> far inside tolerance.
            u_all = singles.tile([128, N_CHUNKS], bf16)

            w_ps = [
                psum_pool.tile([1, 512], f32, tag=f"w{jc}", name=f"w_ps{jc}")
                for jc in range(8)
            ]

            QUART = SEQ_LEN // 4  # 1024

            def emit_phase1(n):
                """DMA + big fused cast/row-sum ops + edge sums for chunk n.
                Returns the tiles phase 2 needs."""
                last = n == N_CHUNKS - 1
                rows = slice(n * 128, (n + 1) * 128)
                xt = xpool.tile([128, SEQ_LEN], f32, tag="x", name=f"xt{n}")
                # Steady state: 2 half-DMAs (big descriptors, low issue
                # overhead). Last chunk: 4 quarter-DMAs so the post-DMA
                # cast latency is one quarter, not a half.
                splits = (
                    [0, QUART, HALF, HALF + QUART, SEQ_LEN]
                    if last
                    else [0, HALF, SEQ_LEN]
                )
                for lo, hi in zip(splits[:-1], splits[1:]):
                    nc.sync.dma_start(out=xt[:, lo:hi], in_=x[rows, lo:hi])

                xb = bfpool.tile([128, SEQ_LEN], bf16, tag="xb", name=f"xb{n}")
                # Fused f32->bf16 cast + per-partition row-sum: ACT takes
                # [0:HALF], DVE [HALF:SEQ_LEN].
                t_a = small.tile([128, 1], f32, tag="ta", name=f"ta{n}")
                t_b = small.tile([128, 1], f32, tag="tb", name=f"tb{n}")
                if last:
                    accs = [
                        small.tile([128, 1], f32, tag=f"q{q}", name=f"acc{q}")
                        for q in range(4)
                    ]
                    for q, acc in enumerate(accs):
                        sl = slice(q * QUART, (q + 1) * QUART)
                        if q < 2:
                            nc.scalar.activation(
                                out=xb[:, sl],
                                in_=xt[:, sl],
                                func=Act.Copy,
                                accum_out=acc[:],
                            )
                        else:
                            nc.vector.tensor_scalar(
                                xb[:, sl],
                                xt[:, sl],
                                1.0,
                                0.0,
                                Alu.mult,
                                Alu.add,
                                accum_out=acc[:],
                            )
                    nc.scalar.activation(
                        out=t_a[:],
                        in_=accs[0][:],
                        func=Act.Identity,
                        bias=accs[1][:],
                    )
                    nc.vector.tensor_tensor(
                        t_b[:], accs[2][:], accs[3][:], Alu.add
                    )
                else:
                    nc.scalar.activation(
                        out=xb[:, 0:HALF],
                        in_=xt[:, 0:HALF],
                        func=Act.Copy,
                        accum_out=t_a[:],
                    )
                    nc.vector.tensor_scalar(
                        xb[:, HALF:SEQ_LEN],
                        xt[:, HALF:SEQ_LEN],
                        1.0,
                        0.0,
                        Alu.mult,
                        Alu.add,
                        accum_out=t_b[:],
                    )

                # Edge corrections: e = sum over edge cols of x * (c - s).
                e_l = small.tile([128, 1], f32, tag="el", name=f"el{n}")
                e_r = small.tile([128, 1], f32, tag="er", name=f"er{n}")
                junk = small.tile([128, EDGE], bf16, tag="junk", name=f"jl{n}")
                nc.vector.scalar_tensor_tensor(
                    junk[:],
                    xt[:, 0:EDGE],
                    1.0,
                    cd_sb[:, 0:EDGE],
                    Alu.mult,
                    Alu.mult,
                    accum_out=e_l[:],
                )
                junk2 = small.tile([128, EDGE], bf16, tag="junk2", name=f"jr{n}")
                nc.vector.scalar_tensor_tensor(
                    junk2[:],
                    xt[:, OUT_LEN:SEQ_LEN],
                    1.0,
                    cd_sb[:, EDGE : 2 * EDGE],
                    Alu.mult,
                    Alu.mult,
                    accum_out=e_r[:],
                )
                return xb, t_a, t_b, e_l, e_r

            def emit_phase2(n, state):
                """Sigmoid chain + u + matmuls for chunk n. Deferred until
                after chunk n+1's phase 1 so these latecomers never block
                the next chunk's big ops in the engine FIFOs."""
                xb, t_a, t_b, e_l, e_r = state
                # logits = (s/out_len)*T + (e/out_len + bias); scale and
                # bias ride the sigmoid ACTIVATE as AP operands.
                e_sum = small.tile([128, 1], f32, tag="esum", name=f"es{n}")
                nc.vector.tensor_tensor(e_sum[:], e_l[:], e_r[:], Alu.add)
                bias_eff = small.tile([128, 1], f32, tag="biaseff", name=f"be{n}")
                nc.vector.tensor_scalar(
                    bias_eff[:],
                    e_sum[:],
                    1.0 / OUT_LEN,
                    par_sb[:, 0:1],
                    Alu.mult,
                    Alu.add,
                )
                t_sum = small.tile([128, 1], f32, tag="tsum", name=f"ts{n}")
                nc.vector.tensor_tensor(t_sum[:], t_a[:], t_b[:], Alu.add)
                sig = small.tile([128, 1], f32, tag="sig", name=f"sg{n}")
                nc.scalar.activation(
                    out=sig[:],
                    in_=t_sum[:],
                    func=Act.Sigmoid,
                    bias=bias_eff[:],
                    scale=par_sb[:, 1:2],
                )
                # u = (sig - y) / out_len, bf16, straight into the matmul
                # stationary column.
                nc.vector.tensor_scalar(
                    u_all[:, n : n + 1],
                    sig[:],
                    y_sb[:, n : n + 1],
                    1.0 / OUT_LEN,
                    Alu.subtract,
                    Alu.mult,
                )
                # w[jc*512:(jc+1)*512] += u^T @ X_chunk  (PSUM accumulate)
                for jc in range(8):
                    nc.tensor.matmul(
                        w_ps[jc][:1, :],
                        u_all[:, n : n + 1],
                        xb[:, jc * 512 : (jc + 1) * 512],
                        start=(n == 0),
                        stop=(n == N_CHUNKS - 1),
                    )

            pending = None
            for n in range(N_CHUNKS):
                if n == N_CHUNKS - 1 and pending is not None:
                    # Chunk n-1's data landed long ago; flush its phase 2
                    # first so the final chunk's tail chain has empty FIFOs.
                    emit_phase2(n - 1, pending)
                    pending = None
                state = emit_phase1(n)
                if pending is not None:
                    emit_phase2(n - 1, pending)
                pending = state
            emit_phase2(N_CHUNKS - 1, pending)

            w_sb = singles.tile([1, SEQ_LEN], f32)
            for jc in range(8):
                dst = w_sb[:, jc * 512 : (jc + 1) * 512]
                if jc % 2 == 0:
                    nc.vector.tensor_copy(dst, w_ps[jc][:1, :])
                else:
                    nc.scalar.copy(dst, w_ps[jc][:1, :])
            nc.sync.dma_start(out=w_out[:], in_=w_sb[:])
            nc.sync.dma_start(out=u_out[:], in_=u_all[:])

    nc.finalize()
    return nc


def _get_nc():
    if "nc" not in _NC_CACHE:
        _NC_CACHE["nc"] = _build_nc()
    return _NC_CACHE["nc"]


def _make_in_maps(x, y, kernel, bias):
    # Host precompute of the tiny per-kernel constants.
    k64 = kernel[0, :, 0].astype(np.float64)
    s = k64.sum()
    c = np.convolve(k64, np.ones(OUT_LEN, np.float64), mode="full")  # (L,)
    cdelta = c - s
    cd_packed = np.concatenate(
        [cdelta[0:EDGE], cdelta[OUT_LEN:SEQ_LEN]]
    ).astype(np.float32)
    cd_full = np.ascontiguousarray(
        np.broadcast_to(cd_packed[None, :], (128, 2 * EDGE))
    )
    par = np.ascontiguousarray(
        np.broadcast_to(
            np.array([bias[0, 0, 0], s / OUT_LEN], np.float32)[None, :],
            (128, 2),
        )
    )

    in_maps = []
    for core in range(N_CORES):
        xs = np.ascontiguousarray(
            x[0, core * B_SHARD : (core + 1) * B_SHARD, :], dtype=np.float32
        )
        ys = y[0, core * B_SHARD : (core + 1) * B_SHARD, 0].astype(np.float32)
        y_t = np.ascontiguousarray(ys.reshape(N_CHUNKS, 128).T)
        in_maps.append({"x": xs, "y_t": y_t, "cd": cd_full, "par": par})
    return in_maps


def _finalize(results, kernel, bias):
    w_total = np.zeros(SEQ_LEN, np.float64)
    u_sum = 0.0
    for res in results:
        w_total += res["w_out"][0].astype(np.float64)
        u_sum += res["u_out"].astype(np.float64).sum()

    cs = np.concatenate([[0.0], np.cumsum(w_total)])
    d_kernel = (cs[OUT_LEN : OUT_LEN + KERNEL_SIZE] - cs[0:KERNEL_SIZE]) / BATCH
    d_bias = OUT_LEN * u_sum / BATCH

    kernel_new = (
        kernel.astype(np.float64) - LR * d_kernel[None, :, None]
    ).astype(np.float32)
    bias_new = (bias.astype(np.float64) - LR * d_bias).astype(np.float32)
    return kernel_new, bias_new


def kernel(x, y, kernel, bias, _trace=False, _result_holder=None):
    from concourse.bass_utils import run_bass_kernel_spmd

    nc = _get_nc()
    in_maps = _make_in_maps(
        np.asarray(x), np.asarray(y), np.asarray(kernel), np.asarray(bias)
    )
    out = run_bass_kernel_spmd(
        nc, in_maps, core_ids=list(range(N_CORES)), trace=_trace
    )
    if _result_holder is not None:
        _result_holder.append(out)
    return _finalize(out.results, np.asarray(kernel), np.asarray(bias))


# revision 16
# speedup vs baseline: 1.0430x; 1.0430x over previous
"""Trainium2 Bass kernel for nn_Conv1DTraining (one SGD step of a pooled
1-D conv + BCE head).

Math (collapsing the reference):
  c = conv_full(kernel, ones(out_len))            # (L,) — constant s=sum(kernel)
                                                  # except 127 edge cols each side
  pooled[b] = sum_j x[b,j]*c[j] = s*T[b] + edge corrections, T[b]=row sum
  u[b]      = (sigmoid(pooled[b]/out_len + bias) - y[b]) / out_len
  w[j]      = sum_b u[b]*x[b,j]                   # the only other O(B*L) pass
  d_kernel[k] = window_sum(w, k..k+out_len-1)/B ; d_bias = out_len*sum(u)/B

Sharding: data-parallel over batch (8192 -> 8 x 1024). Each core streams its
16 MiB x-shard once (DMA-bound), computing T on ACT/DVE (fused f32->bf16 cast
+ row-sum via accum_out) and w on the PE (u^T @ X, bf16, PSUM-accumulated).
Tiny per-core partials (w: 16 KB, u: 4 KB) are reduced on the host, where the
window sums and the SGD update are trivial.
"""

import numpy as np

KERNEL_SIZE = 128
SEQ_LEN = 4096
BATCH = 8192
OUT_LEN = SEQ_LEN - KERNEL_SIZE + 1  # 3969
LR = 1.0
N_CORES = 8
B_SHARD = BATCH // N_CORES  # 1024
N_CHUNKS = B_SHARD // 128  # 8
EDGE = KERNEL_SIZE - 1  # 127
HALF = SEQ_LEN // 2  # 2048

_NC_CACHE = {}


def _build_nc():
    import concourse.bacc as bacc
    import concourse.tile as tile
    from concourse import mybir

    f32 = mybir.dt.float32
    bf16 = mybir.dt.bfloat16
    Alu = mybir.AluOpType
    Act = mybir.ActivationFunctionType

    nc = bacc.Bacc(None, target_bir_lowering=False)

    x = nc.dram_tensor("x", [B_SHARD, SEQ_LEN], f32, kind="ExternalInput")
    y_t = nc.dram_tensor("y_t", [128, N_CHUNKS], f32, kind="ExternalInput")
    cd = nc.dram_tensor("cd", [128, 2 * EDGE], f32, kind="ExternalInput")
    par = nc.dram_tensor("par", [128, 2], f32, kind="ExternalInput")
    w_out = nc.dram_tensor("w_out", [1, SEQ_LEN], f32, kind="ExternalOutput")
    u_out = nc.dram_tensor("u_out", [128, N_CHUNKS], bf16, kind="ExternalOutput")

    with tile.TileContext(nc) as tc:
        with (
            tc.tile_pool(name="singles", bufs=1) as singles,
            tc.tile_pool(name="xpool", bufs=3) as xpool,
            tc.tile_pool(name="bfpool", bufs=3) as bfpool,
            tc.tile_pool(name="small", bufs=4) as small,
            tc.tile_pool(name="psum", bufs=1, space="PSUM") as psum_pool,
        ):
            # Constants ride SWDGE (gpsimd) so the Sync HWDGE ring is
            # dedicated to the big x stream from instruction 0.
            y_sb = singles.tile([128, N_CHUNKS], f32)
            nc.gpsimd.dma_start(out=y_sb[:], in_=y_t[:])
            cd_sb = singles.tile([128, 2 * EDGE], f32)
            nc.gpsimd.dma_start(out=cd_sb[:], in_=cd[:])
            par_sb = singles.tile([128, 2], f32)
            nc.gpsimd.dma_start(out=par_sb[:], in_=par[:])
            # u in bf16 doubles as the matmul stationary operand; the host
            # only needs sum(u), where bf16 noise is ~0.4% of a gradient
            # that is itself ~5% of bias -> far inside tolerance.
            u_all = singles.tile([128, N_CHUNKS], bf16)

            w_ps = [
                psum_pool.tile([1, 512], f32, tag=f"w{jc}", name=f"w_ps{jc}")
                for jc in range(8)
            ]

            QUART = SEQ_LEN // 4  # 1024

            def emit_phase1(n):
                """DMA + big fused cast/row-sum ops + edge sums for chunk n.
                Returns the tiles phase 2 needs."""
                last = n == N_CHUNKS - 1
                rows = slice(n * 128, (n + 1) * 128)
                xt = xpool.tile([128, SEQ_LEN], f32, tag="x", name=f"xt{n}")
                # Steady state: 2 half-DMAs (big descriptors, low issue
                # overhead). Last chunk: 4 quarter-DMAs so the post-DMA
                # cast latency is one quarter, not a half.
                splits = (
                    [0, QUART, HALF, HALF + QUART, SEQ_LEN]
                    if last
                    else [0, HALF, SEQ_LEN]
                )
                for lo, hi in zip(splits[:-1], splits[1:]):
                    nc.sync.dma_start(out=xt[:, lo:hi], in_=x[rows, lo:hi])

                xb = bfpool.tile([128, SEQ_LEN], bf16, tag="xb", name=f"xb{n}")
                # Fused f32->bf16 cast + per-partition row-sum: ACT takes
                # [0:HALF], DVE [HALF:SEQ_LEN].
                t_a = small.tile([128, 1], f32, tag="ta", name=f"ta{n}")
                t_b = small.tile([128, 1], f32, tag="tb", name=f"tb{n}")
                if last:
                    accs = [
                        small.tile([128, 1], f32, tag=f"q{q}", name=f"acc{q}")
                        for q in range(4)
                    ]
                    for q, acc in enumerate(accs):
                        sl = slice(q * QUART, (q + 1) * QUART)
                        if q < 2:
                            nc.scalar.activation(
                                out=xb[:, sl],
                                in_=xt[:, sl],
                                func=Act.Copy,
                                accum_out=acc[:],
                            )
                        else:
                            nc.vector.tensor_scalar(
                                xb[:, sl],
                                xt[:, sl],
                                1.0,
                                0.0,
                                Alu.mult,
                                Alu.add,
                                accum_out=acc[:],
                            )
                    nc.scalar.activation(
                        out=t_a[:],
                        in_=accs[0][:],
                        func=Act.Identity,
                        bias=accs[1][:],
                    )
                    nc.vector.tensor_tensor(
                        t_b[:], accs[2][:], accs[3][:], Alu.add
                    )
                else:
                    nc.scalar.activation(
                        out=xb[:, 0:HALF],
                        in_=xt[:, 0:HALF],
                        func=Act.Copy,
                        accum_out=t_a[:],
                    )
                    nc.vector.tensor_scalar(
                        xb[:, HALF:SEQ_LEN],
                        xt[:, HALF:SEQ_LEN],
                        1.0,
                        0.0,
                        Alu.mult,
                        Alu.add,
                        accum_out=t_b[:],
                    )

                # Edge corrections: e = sum over edge cols of x * (c - s).
                e_l = small.tile([128, 1], f32, tag="el", name=f"el{n}")
                e_r = small.tile([128, 1], f32, tag="er", name=f"er{n}")
                junk = small.tile([128, EDGE], bf16, tag="junk", name=f"jl{n}")
                nc.vector.scalar_tensor_tensor(
                    junk[:],
                    xt[:, 0:EDGE],
                    1.0,
                    cd_sb[:, 0:EDGE],
                    Alu.mult,
                    Alu.mult,
                    accum_out=e_l[:],
                )
                junk2 = small.tile([128, EDGE], bf16, tag="junk2", name=f"jr{n}")
                nc.vector.scalar_tensor_tensor(
                    junk2[:],
                    xt[:, OUT_LEN:SEQ_LEN],
                    1.0,
                    cd_sb[:, EDGE : 2 * EDGE],
                    Alu.mult,
                    Alu.mult,
                    accum_out=e_r[:],
                )
                return xb, t_a, t_b, e_l, e_r

            def emit_phase2(n, state):
                """Sigmoid chain + u + matmuls for chunk n. Deferred until
                after chunk n+1's phase 1 so these latecomers never block
                the next chunk's big ops in the engine FIFOs."""
                xb, t_a, t_b, e_l, e_r = state
                # logits = (s/out_len)*T + (e/out_len + bias); scale and
                # bias ride the sigmoid ACTIVATE as AP operands.
                e_sum = small.tile([128, 1], f32, tag="esum", name=f"es{n}")
                nc.vector.tensor_tensor(e_sum[:], e_l[:], e_r[:], Alu.add)
                bias_eff = small.tile([128, 1], f32, tag="biaseff", name=f"be{n}")
                nc.vector.tensor_scalar(
                    bias_eff[:],
                    e_sum[:],
                    1.0 / OUT_LEN,
                    par_sb[:, 0:1],
                    Alu.mult,
                    Alu.add,
                )
                t_sum = small.tile([128, 1], f32, tag="tsum", name=f"ts{n}")
                nc.vector.tensor_tensor(t_sum[:], t_a[:], t_b[:], Alu.add)
                sig = small.tile([128, 1], f32, tag="sig", name=f"sg{n}")
                nc.scalar.activation(
                    out=sig[:],
                    in_=t_sum[:],
                    func=Act.Sigmoid,
                    bias=bias_eff[:],
                    scale=par_sb[:, 1:2],
                )
                # u = (sig - y) / out_len, bf16, straight into the matmul
                # stationary column.
                nc.vector.tensor_scalar(
                    u_all[:, n : n + 1],
                    sig[:],
                    y_sb[:, n : n + 1],
                    1.0 / OUT_LEN,
                    Alu.subtract,
                    Alu.mult,
                )
                # w[jc*512:(jc+1)*512] += u^T @ X_chunk  (PSUM accumulate)
                for jc in range(8):
                    nc.tensor.matmul(
                        w_ps[jc][:1, :],
                        u_all[:, n : n + 1],
                        xb[:, jc * 512 : (jc + 1) * 512],
                        start=(n == 0),
                        stop=(n == N_CHUNKS - 1),
                    )

            for n in range(N_CHUNKS):
                emit_phase2(n, emit_phase1(n))

            w_sb = singles.tile([1, SEQ_LEN], f32)
            for jc in range(8):
                dst = w_sb[:, jc * 512 : (jc + 1) * 512]
                if jc % 2 == 0:
                    nc.vector.tensor_copy(dst, w_ps[jc][:1, :])
                else:
                    nc.scalar.copy(dst, w_ps[jc][:1, :])
            nc.sync.dma_start(out=w_out[:], in_=w_sb[:])
            nc.sync.dma_start(out=u_out[:], in_=u_all[:])

    nc.finalize()
    return nc


def _get_nc():
    if "nc" not in _NC_CACHE:
        _NC_CACHE["nc"] = _build_nc()
    return _NC_CACHE["nc"]


def _make_in_maps(x, y, kernel, bias):
    # Host precompute of the tiny per-kernel constants.
    k64 = kernel[0, :, 0].astype(np.float64)
    s = k64.sum()
    c = np.convolve(k64, np.ones(OUT_LEN, np.float64), mode="full")  # (L,)
    cdelta = c - s
    cd_packed = np.concatenate(
        [cdelta[0:EDGE], cdelta[OUT_LEN:SEQ_LEN]]
    ).astype(np.float32)
    cd_full = np.ascontiguousarray(
        np.broadcast_to(cd_packed[None, :], (128, 2 * EDGE))
    )
    par = np.ascontiguousarray(
        np.broadcast_to(
            np.array([bias[0, 0, 0], s / OUT_LEN], np.float32)[None, :],
            (128, 2),
        )
    )

    in_maps = []
    for core in range(N_CORES):
        xs = np.ascontiguousarray(
            x[0, core * B_SHARD : (core + 1) * B_SHARD, :], dtype=np.float32
        )
        ys = y[0, core * B_SHARD : (core + 1) * B_SHARD, 0].astype(np.float32)
        y_t = np.ascontiguousarray(ys.reshape(N_CHUNKS, 128).T)
        in_maps.append({"x": xs, "y_t": y_t, "cd": cd_full, "par": par})
    return in_maps


def _finalize(results, kernel, bias):
    w_total = np.zeros(SEQ_LEN, np.float64)
    u_sum = 0.0
    for res in results:
        w_total += res["w_out"][0].astype(np.float64)
        u_sum += res["u_out"].astype(np.float64).sum()

    cs = np.concatenate([[0.0], np.cumsum(w_total)])
    d_kernel = (cs[OUT_LEN : OUT_LEN + KERNEL_SIZE] - cs[0:KERNEL_SIZE]) / BATCH
    d_bias = OUT_LEN * u_sum / BATCH

    kernel_new = (
        kernel.astype(np.float64) - LR * d_kernel[None, :, None]
    ).astype(np.float32)
    bias_new = (bias.astype(np.float64) - LR * d_bias).astype(np.float32)
    return kernel_new, bias_new


def kernel(x, y, kernel, bias, _trace=False, _result_holder=None):
    from concourse.bass_utils import run_bass_kernel_spmd

    nc = _get_nc()
    in_maps = _make_in_maps(
        np.asarray(x), np.asarray(y), np.asarray(kernel), np.asarray(bias)
    )
    out = run_bass_kernel_spmd(
        nc, in_maps, core_ids=list(range(N_CORES)), trace=_trace
    )
    if _result_holder is not None:
        _result_holder.append(out)
    return _finalize(out.results, np.asarray(kernel), np.asarray(bias))


# revision 17
# speedup vs baseline: 1.1580x; 1.1102x over previous
"""Trainium2 Bass kernel for nn_Conv1DTraining (one SGD step of a pooled
1-D conv + BCE head).

Math (collapsing the reference):
  c = conv_full(kernel, ones(out_len))            # (L,) — constant s=sum(kernel)
                                                  # except 127 edge cols each side
  pooled[b] = sum_j x[b,j]*c[j] = s*T[b] + edge corrections, T[b]=row sum
  u[b]      = (sigmoid(pooled[b]/out_len + bias) - y[b]) / out_len
  w[j]      = sum_b u[b]*x[b,j]                   # the only other O(B*L) pass
  d_kernel[k] = window_sum(w, k..k+out_len-1)/B ; d_bias = out_len*sum(u)/B

Sharding: data-parallel over batch (8192 -> 8 x 1024). Each core streams its
16 MiB x-shard once (DMA-bound), computing T on ACT/DVE (fused f32->bf16 cast
+ row-sum via accum_out) and w on the PE (u^T @ X, bf16, PSUM-accumulated).
Tiny per-core partials (w: 16 KB, u: 4 KB) are reduced on the host, where the
window sums and the SGD update are trivial.
"""

import numpy as np

KERNEL_SIZE = 128
SEQ_LEN = 4096
BATCH = 8192
OUT_LEN = SEQ_LEN - KERNEL_SIZE + 1  # 3969
LR = 1.0
N_CORES = 8
B_SHARD = BATCH // N_CORES  # 1024
N_CHUNKS = B_SHARD // 128  # 8
EDGE = KERNEL_SIZE - 1  # 127
HALF = SEQ_LEN // 2  # 2048

_NC_CACHE = {}


def _build_nc():
    import concourse.bacc as bacc
    import concourse.tile as tile
    from concourse import mybir

    f32 = mybir.dt.float32
    bf16 = mybir.dt.bfloat16
    Alu = mybir.AluOpType
    Act = mybir.ActivationFunctionType

    nc = bacc.Bacc(None, target_bir_lowering=False)

    x = nc.dram_tensor("x", [B_SHARD, SEQ_LEN], f32, kind="ExternalInput")
    y_t = nc.dram_tensor("y_t", [128, N_CHUNKS], f32, kind="ExternalInput")
    cd = nc.dram_tensor("cd", [128, 2 * EDGE], f32, kind="ExternalInput")
    par = nc.dram_tensor("par", [128, 2], f32, kind="ExternalInput")
    w_out = nc.dram_tensor("w_out", [1, SEQ_LEN], f32, kind="ExternalOutput")
    u_out = nc.dram_tensor("u_out", [128, N_CHUNKS], bf16, kind="ExternalOutput")

    with tile.TileContext(nc) as tc:
        with (
            tc.tile_pool(name="singles", bufs=1) as singles,
            tc.tile_pool(name="xpool", bufs=3) as xpool,
            tc.tile_pool(name="bfpool", bufs=3) as bfpool,
            tc.tile_pool(name="small", bufs=4) as small,
            tc.tile_pool(name="psum", bufs=1, space="PSUM") as psum_pool,
        ):
            # Constants ride SWDGE (gpsimd) so the Sync HWDGE ring is
            # dedicated to the big x stream from instruction 0.
            y_sb = singles.tile([128, N_CHUNKS], f32)
            nc.gpsimd.dma_start(out=y_sb[:], in_=y_t[:])
            cd_sb = singles.tile([128, 2 * EDGE], f32)
            nc.gpsimd.dma_start(out=cd_sb[:], in_=cd[:])
            par_sb = singles.tile([128, 2], f32)
            nc.gpsimd.dma_start(out=par_sb[:], in_=par[:])
            # u in bf16 doubles as the matmul stationary operand; the host
            # only needs sum(u), where bf16 noise is ~0.4% of a gradient
            # that is itself ~5% of bias -> far inside tolerance.
            u_all = singles.tile([128, N_CHUNKS], bf16)

            w_ps = [
                psum_pool.tile([1, 512], f32, tag=f"w{jc}", name=f"w_ps{jc}")
                for jc in range(8)
            ]

            QUART = SEQ_LEN // 4  # 1024

            def emit_phase1(n):
                """DMA + big fused cast/row-sum ops + edge sums for chunk n.
                Returns the tiles phase 2 needs."""
                last = n == N_CHUNKS - 1
                rows = slice(n * 128, (n + 1) * 128)
                xt = xpool.tile([128, SEQ_LEN], f32, tag="x", name=f"xt{n}")
                # Steady state: one full-chunk DMA (16 KB contiguous per
                # partition — longest descriptors the layout allows, best
                # HBM efficiency under 8-core contention). Last chunk:
                # 4 quarter-DMAs so the post-DMA cast latency is short.
                splits = (
                    [0, QUART, HALF, HALF + QUART, SEQ_LEN]
                    if last
                    else [0, SEQ_LEN]
                )
                for lo, hi in zip(splits[:-1], splits[1:]):
                    nc.sync.dma_start(out=xt[:, lo:hi], in_=x[rows, lo:hi])

                xb = bfpool.tile([128, SEQ_LEN], bf16, tag="xb", name=f"xb{n}")
                # Fused f32->bf16 cast + per-partition row-sum: ACT takes
                # [0:HALF], DVE [HALF:SEQ_LEN].
                t_a = small.tile([128, 1], f32, tag="ta", name=f"ta{n}")
                t_b = small.tile([128, 1], f32, tag="tb", name=f"tb{n}")
                if last:
                    accs = [
                        small.tile([128, 1], f32, tag=f"q{q}", name=f"acc{q}")
                        for q in range(4)
                    ]
                    for q, acc in enumerate(accs):
                        sl = slice(q * QUART, (q + 1) * QUART)
                        if q < 2:
                            nc.scalar.activation(
                                out=xb[:, sl],
                                in_=xt[:, sl],
                                func=Act.Copy,
                                accum_out=acc[:],
                            )
                        else:
                            nc.vector.tensor_scalar(
                                xb[:, sl],
                                xt[:, sl],
                                1.0,
                                0.0,
                                Alu.mult,
                                Alu.add,
                                accum_out=acc[:],
                            )
                    nc.scalar.activation(
                        out=t_a[:],
                        in_=accs[0][:],
                        func=Act.Identity,
                        bias=accs[1][:],
                    )
                    nc.vector.tensor_tensor(
                        t_b[:], accs[2][:], accs[3][:], Alu.add
                    )
                else:
                    nc.scalar.activation(
                        out=xb[:, 0:HALF],
                        in_=xt[:, 0:HALF],
                        func=Act.Copy,
                        accum_out=t_a[:],
                    )
                    nc.vector.tensor_scalar(
                        xb[:, HALF:SEQ_LEN],
                        xt[:, HALF:SEQ_LEN],
                        1.0,
                        0.0,
                        Alu.mult,
                        Alu.add,
                        accum_out=t_b[:],
                    )

                # Edge corrections: e = sum over edge cols of x * (c - s).
                e_l = small.tile([128, 1], f32, tag="el", name=f"el{n}")
                e_r = small.tile([128, 1], f32, tag="er", name=f"er{n}")
                junk = small.tile([128, EDGE], bf16, tag="junk", name=f"jl{n}")
                nc.vector.scalar_tensor_tensor(
                    junk[:],
                    xt[:, 0:EDGE],
                    1.0,
                    cd_sb[:, 0:EDGE],
                    Alu.mult,
                    Alu.mult,
                    accum_out=e_l[:],
                )
                junk2 = small.tile([128, EDGE], bf16, tag="junk2", name=f"jr{n}")
                nc.vector.scalar_tensor_tensor(
                    junk2[:],
                    xt[:, OUT_LEN:SEQ_LEN],
                    1.0,
                    cd_sb[:, EDGE : 2 * EDGE],
                    Alu.mult,
                    Alu.mult,
                    accum_out=e_r[:],
                )
                return xb, t_a, t_b, e_l, e_r

            def emit_phase2(n, state):
                """Sigmoid chain + u + matmuls for chunk n. Deferred until
                after chunk n+1's phase 1 so these latecomers never block
                the next chunk's big ops in the engine FIFOs."""
                xb, t_a, t_b, e_l, e_r = state
                # logits = (s/out_len)*T + (e/out_len + bias); scale and
                # bias ride the sigmoid ACTIVATE as AP operands.
                e_sum = small.tile([128, 1], f32, tag="esum", name=f"es{n}")
                nc.vector.tensor_tensor(e_sum[:], e_l[:], e_r[:], Alu.add)
                bias_eff = small.tile([128, 1], f32, tag="biaseff", name=f"be{n}")
                nc.vector.tensor_scalar(
                    bias_eff[:],
                    e_sum[:],
                    1.0 / OUT_LEN,
                    par_sb[:, 0:1],
                    Alu.mult,
                    Alu.add,
                )
                t_sum = small.tile([128, 1], f32, tag="tsum", name=f"ts{n}")
                nc.vector.tensor_tensor(t_sum[:], t_a[:], t_b[:], Alu.add)
                sig = small.tile([128, 1], f32, tag="sig", name=f"sg{n}")
                nc.scalar.activation(
                    out=sig[:],
                    in_=t_sum[:],
                    func=Act.Sigmoid,
                    bias=bias_eff[:],
                    scale=par_sb[:, 1:2],
                )
                # u = (sig - y) / out_len, bf16, straight into the matmul
                # stationary column.
                nc.vector.tensor_scalar(
                    u_all[:, n : n + 1],
                    sig[:],
                    y_sb[:, n : n + 1],
                    1.0 / OUT_LEN,
                    Alu.subtract,
                    Alu.mult,
                )
                # w[jc*512:(jc+1)*512] += u^T @ X_chunk  (PSUM accumulate)
                for jc in range(8):
                    nc.tensor.matmul(
                        w_ps[jc][:1, :],
                        u_all[:, n : n + 1],
                        xb[:, jc * 512 : (jc + 1) * 512],
                        start=(n == 0),
                        stop=(n == N_CHUNKS - 1),
                    )

            for n in range(N_CHUNKS):
                emit_phase2(n, emit_phase1(n))

            w_sb = singles.tile([1, SEQ_LEN], f32)
            for jc in range(8):
                dst = w_sb[:, jc * 512 : (jc + 1) * 512]
                if jc % 2 == 0:
                    nc.vector.tensor_copy(dst, w_ps[jc][:1, :])
                else:
                    nc.scalar.copy(dst, w_ps[jc][:1, :])
            nc.sync.dma_start(out=w_out[:], in_=w_sb[:])
            nc.sync.dma_start(out=u_out[:], in_=u_all[:])

    nc.finalize()
    return nc


def _get_nc():
    if "nc" not in _NC_CACHE:
        _NC_CACHE["nc"] = _build_nc()
    return _NC_CACHE["nc"]


def _make_in_maps(x, y, kernel, bias):
    # Host precompute of the tiny per-kernel constants.
    k64 = kernel[0, :, 0].astype(np.float64)
    s = k64.sum()
    c = np.convolve(k64, np.ones(OUT_LEN, np.float64), mode="full")  # (L,)
    cdelta = c - s
    cd_packed = np.concatenate(
        [cdelta[0:EDGE], cdelta[OUT_LEN:SEQ_LEN]]
    ).astype(np.float32)
    cd_full = np.ascontiguousarray(
        np.broadcast_to(cd_packed[None, :], (128, 2 * EDGE))
    )
    par = np.ascontiguousarray(
        np.broadcast_to(
            np.array([bias[0, 0, 0], s / OUT_LEN], np.float32)[None, :],
            (128, 2),
        )
    )

    in_maps = []
    for core in range(N_CORES):
        xs = np.ascontiguousarray(
            x[0, core * B_SHARD : (core + 1) * B_SHARD, :], dtype=np.float32
        )
        ys = y[0, core * B_SHARD : (core + 1) * B_SHARD, 0].astype(np.float32)
        y_t = np.ascontiguousarray(ys.reshape(N_CHUNKS, 128).T)
        in_maps.append({"x": xs, "y_t": y_t, "cd": cd_full, "par": par})
    return in_maps


def _finalize(results, kernel, bias):
    w_total = np.zeros(SEQ_LEN, np.float64)
    u_sum = 0.0
    for res in results:
        w_total += res["w_out"][0].astype(np.float64)
        u_sum += res["u_out"].astype(np.float64).sum()

    cs = np.concatenate([[0.0], np.cumsum(w_total)])
    d_kernel = (cs[OUT_LEN : OUT_LEN + KERNEL_SIZE] - cs[0:KERNEL_SIZE]) / BATCH
    d_bias = OUT_LEN * u_sum / BATCH

    kernel_new = (
        kernel.astype(np.float64) - LR * d_kernel[None, :, None]
    ).astype(np.float32)
    bias_new = (bias.astype(np.float64) - LR * d_bias).astype(np.float32)
    return kernel_new, bias_new


def kernel(x, y, kernel, bias, _trace=False, _result_holder=None):
    from concourse.bass_utils import run_bass_kernel_spmd

    nc = _get_nc()
    in_maps = _make_in_maps(
        np.asarray(x), np.asarray(y), np.asarray(kernel), np.asarray(bias)
    )
    out = run_bass_kernel_spmd(
        nc, in_maps, core_ids=list(range(N_CORES)), trace=_trace
    )
    if _result_holder is not None:
        _result_holder.append(out)
    return _finalize(out.results, np.asarray(kernel), np.asarray(bias))
